# revision 6
# baseline (speedup 1.0000x reference)
"""Permutohedral submanifold convolution on 8 Trainium2 cores.

out[n] = bias + sum_k W[k]^T @ features[neighbor_idx[k, n]]
N=2M sites, K=13 offsets, C=16 channels.

Strategy: shard sites across 8 cores (250k each); replicate the feature
table (indices are unstructured, so any site may need any row); per core,
gather neighbor rows with big indirect DMAs, transpose gathered blocks on
TensorE so the (k,c)=208 contraction lands on partitions, then accumulate
two matmuls (k=0..7 -> K=128, k=8..12 -> K=80) into PSUM per 512 sites.
Bias is fused into the PSUM->SBUF copy on the scalar engine. Output is
written transposed [16, NS] and untransposed on the host.
"""

import numpy as np
from contextlib import ExitStack

import concourse.tile as tile
from concourse import bacc, bass, mybir
from concourse._compat import with_exitstack
from concourse.bass_utils import run_bass_kernel_spmd
from concourse.masks import make_identity

N_SITES = 2_000_000
C = 16
K = 13
CORES = 8
PER_CORE = N_SITES // CORES        # 250_000
T = 32                             # site-tiles (of 128 sites) per batch
BATCH = 128 * T                    # 4096 sites per gather batch
NB = (PER_CORE + BATCH - 1) // BATCH   # 62
NS_PAD = NB * BATCH                # 253_952
FPT = K * C                        # 208 gathered floats per site
GROUPS = T // 4                    # groups of 512 sites per batch

FP32 = mybir.dt.float32
INT32 = mybir.dt.int32

TRACE = False
TRACE_CORES = [0]
LAST_EXEC_NS = None
LAST_RESULTS = None

_PROGRAM = None


@with_exitstack
def _conv_kernel(ctx: ExitStack, tc: tile.TileContext,
                 feat, w8, w5, biasv, idx, outT):
    nc = tc.nc

    const = ctx.enter_context(tc.tile_pool(name="const", bufs=1))
    ident = const.tile([128, 128], FP32)
    make_identity(nc, ident[:])
    w8_t = const.tile([128, C], FP32)
    nc.sync.dma_start(out=w8_t[:], in_=w8)
    w5_t = const.tile([80, C], FP32)
    nc.sync.dma_start(out=w5_t[:], in_=w5)
    bias_t = const.tile([C, 1], FP32)
    nc.sync.dma_start(out=bias_t[:], in_=biasv)

    idx_pool = ctx.enter_context(tc.tile_pool(name="idxp", bufs=2))
    x_pool = ctx.enter_context(tc.tile_pool(name="xp", bufs=2))
    xta_pool = ctx.enter_context(tc.tile_pool(name="xta", bufs=3))
    xtb_pool = ctx.enter_context(tc.tile_pool(name="xtb", bufs=3))
    o_pool = ctx.enter_context(tc.tile_pool(name="op", bufs=3))
    psa = ctx.enter_context(tc.tile_pool(name="psa", bufs=2, space="PSUM"))
    psb = ctx.enter_context(tc.tile_pool(name="psb", bufs=2, space="PSUM"))
    pso = ctx.enter_context(tc.tile_pool(name="pso", bufs=2, space="PSUM"))

    for b in range(NB):
        idx_t = idx_pool.tile([128, T * K], INT32)
        nc.sync.dma_start(out=idx_t[:], in_=idx[b])
        x_t = x_pool.tile([128, T * FPT], FP32)
        # x_t[p, m*16:(m+1)*16] = feat[idx_t[p, m], :]
        for m in range(T * K):
            nc.gpsimd.indirect_dma_start(
                out=x_t[:, m * C:(m + 1) * C],
                out_offset=None,
                in_=feat,
                in_offset=bass.IndirectOffsetOnAxis(ap=idx_t[:, m:m + 1], axis=0),
            )
        for g in range(GROUPS):
            xa_ps = psa.tile([128, 512], FP32)
            xb_ps = psb.tile([80, 512], FP32)
            for u in range(4):
                t = g * 4 + u
                base = t * FPT
                nc.tensor.transpose(out=xa_ps[:, u * 128:(u + 1) * 128],
                                    in_=x_t[:, base:base + 128],
                                    identity=ident[:])
                nc.tensor.transpose(out=xb_ps[:, u * 128:(u + 1) * 128],
                                    in_=x_t[:, base + 128:base + FPT],
                                    identity=ident[:])
            xa_sb = xta_pool.tile([128, 512], FP32)
            nc.vector.tensor_copy(out=xa_sb[:], in_=xa_ps[:])
            xb_sb = xtb_pool.tile([80, 512], FP32)
            nc.scalar.copy(out=xb_sb[:], in_=xb_ps[:])
            o_ps = pso.tile([C, 512], FP32)
            nc.tensor.matmul(out=o_ps[:], lhsT=w8_t[:], rhs=xa_sb[:],
                             start=True, stop=False)
            nc.tensor.matmul(out=o_ps[:], lhsT=w5_t[:], rhs=xb_sb[:],
                             start=False, stop=True)
            o_sb = o_pool.tile([C, 512], FP32)
            nc.scalar.add(out=o_sb[:], in_=o_ps[:], add=bias_t[:])
            col = b * BATCH + g * 512
            nc.sync.dma_start(out=outT[:, col:col + 512], in_=o_sb[:])


def _build_program():
    global _PROGRAM
    if _PROGRAM is not None:
        return _PROGRAM
    nc = bacc.Bacc("TRN2", target_bir_lowering=False, debug=True)
    feat = nc.declare_dram_parameter("features", [N_SITES, C], FP32, isOutput=False)
    w8 = nc.declare_dram_parameter("w8", [128, C], FP32, isOutput=False)
    w5 = nc.declare_dram_parameter("w5", [80, C], FP32, isOutput=False)
    biasv = nc.declare_dram_parameter("biasv", [C, 1], FP32, isOutput=False)
    idx = nc.declare_dram_parameter("idx", [NB, 128, T * K], INT32, isOutput=False)
    outT = nc.declare_dram_parameter("outT", [C, NS_PAD], FP32, isOutput=True)
    with tile.TileContext(nc) as tc:
        _conv_kernel(tc, feat[:], w8[:], w5[:], biasv[:], idx[:], outT[:])
    nc.compile()
    _PROGRAM = nc
    return nc


def kernel(features, weight, bias, neighbor_idx):
    features = np.ascontiguousarray(features, dtype=np.float32)
    w = weight.astype(np.float32).reshape(K, C, C)        # [k, c, d]
    w8 = np.ascontiguousarray(w[:8].reshape(128, C))
    w5 = np.ascontiguousarray(w[8:].reshape(80, C))
    biasv = np.ascontiguousarray(bias.astype(np.float32).reshape(C, 1))

    in_maps = []
    for c in range(CORES):
        sl = neighbor_idx[:, c * PER_CORE:(c + 1) * PER_CORE]
        pad = np.zeros((K, NS_PAD), np.int32)
        pad[:, :PER_CORE] = sl
        # site within core = b*4096 + t*128 + p ; free layout t*13+j
        arr = pad.reshape(K, NB, T, 128).transpose(1, 3, 2, 0)
        arr = np.ascontiguousarray(arr).reshape(NB, 128, T * K)
        in_maps.append({
            "features": features,
            "w8": w8,
            "w5": w5,
            "biasv": biasv,
            "idx": arr,
        })

    nc = _build_program()
    res = run_bass_kernel_spmd(
        nc, in_maps, list(range(CORES)),
        trace=TRACE, trace_cores=TRACE_CORES if TRACE else None,
    )
    global LAST_EXEC_NS, LAST_RESULTS
    LAST_RESULTS = res
    LAST_EXEC_NS = res.exec_time_ns

    out = np.empty((N_SITES, C), np.float32)
    for c in range(CORES):
        outT = res.results[c]["outT"]
        out[c * PER_CORE:(c + 1) * PER_CORE] = outT[:, :PER_CORE].T
    return out
